# revision 10
# baseline (speedup 1.0000x reference)
"""AttentionBlock (GroupNorm + MHSA + proj + residual) on 8 Trainium2 cores.

Sharding: pure data-parallel over batch (B=8, one batch element per core).
Per-core dataflow (C=512, N=H*W=1024, 8 heads, hd=64, 32 groups):
  1. GroupNorm stats via bn_stats per channel + PE group-mix matmul,
     affine folded with norm_w/norm_b -> xn (bf16).
  2. qkv: q,k via W^T matmul (q pre-scaled by hd^-0.5 on host);
     V^T computed directly as xn^T @ wv^T so the PV matmul needs no
     transposes on the critical path.
  3. Scores computed TRANSPOSED: S^T[m,n] = K^T Q (row-packed head
     pairs), exp on ACT (psum->sbuf bf16, 4-bank reads).
  4. O' = V^T-stationary matmul over P'^T (col-packed head pairs);
     softmax denominators ride as concurrent M=1 ones-matmuls into a
     shared psum bank (4-way col-tiled per quad).
  5. 1/s via DVE reciprocal_approx_accurate; partition-broadcast via a
     constant selection matmul; normalize during O' psum exit.
  6. proj matmul + bias + residual fused in one scalar_tensor_tensor.
"""

import numpy as np
import ml_dtypes

import concourse.bass as bass
import concourse.tile as tile
from concourse import mybir
from concourse.bass_utils import run_bass_kernel_spmd
from concourse.vector_clock import ScopedClock, VectorClock

f32 = mybir.dt.float32
bf16 = mybir.dt.bfloat16
BF16 = ml_dtypes.bfloat16

_RECIP_MODE = "plain"
B, C, N = 8, 512, 1024
NH, HD, G = 8, 64, 32
EPS = 1e-5
CC = C // 128  # 4 channel chunks
OC_QK = 8      # q+k output chunks (1024 rows)
NC = 2         # n in two 512-windows
MC = 8         # m in eight 128-chunks


def _split_multi_waits(bir_json):
    """This container's walrus build encodes at most one sync-wait command
    per TPB instruction. Engines execute in program order, so any extra
    waits can ride on NoOp instructions inserted immediately before the
    original instruction on the same engine (strictly more conservative
    ordering, semantics preserved)."""
    import orjson

    m = orjson.loads(bir_json)
    nop_id = [0]
    for fn in m.get("functions", []):
        for bb in fn.get("blocks", []):
            insts = bb.get("instructions", [])
            out = []
            for ins in insts:
                si = ins.get("sync_info") or {}
                waits = si.get("on_wait") or []
                eng = ins.get("engine", "Unassigned")
                if len(waits) > 1 and eng != "Unassigned":
                    for w in waits[:-1]:
                        nop_id[0] += 1
                        out.append(
                            {
                                "debug": ins.get("debug", 0),
                                "engine": eng,
                                "ins": [],
                                "outs": [],
                                "name": f"{ins['name']}-w{nop_id[0]}",
                                "opcode": "NoOp",
                                "sync_info": {"on_wait": [w]},
                            }
                        )
                    si = dict(si)
                    si["on_wait"] = [waits[-1]]
                    ins = dict(ins)
                    ins["sync_info"] = si
                out.append(ins)
            bb["instructions"] = out
    return orjson.dumps(m)


def _patch_tile():
    """This container's walrus accepts few sem-waits per instruction; split
    TileContext's kernel-tail drain into one drain per pending proc and
    hoist any remaining multi-waits onto NoOps at compile time."""
    if getattr(tile.TileContext, "_drain_split_patched", False):
        return

    from concourse import bass2jax, bass_utils

    orig_compile = bass_utils.compile_bir_kernel

    def compile_with_split(bir_json, tmpdir, neff_name="file.neff"):
        return orig_compile(_split_multi_waits(bir_json), tmpdir, neff_name=neff_name)

    bass_utils.compile_bir_kernel = compile_with_split
    bass2jax.compile_bir_kernel = compile_with_split

    def _drain_and_barrier_split(self, tick_clock, wait_clock):
        gc = tick_clock.global_clock
        ticks = list(gc)
        for p, t in enumerate(ticks):
            if t <= 0:
                continue
            vec = [0] * len(ticks)
            vec[p] = t
            drain_inst = self.nc.sync.drain()
            wait_clock.add_sem_waits(
                drain_inst.ins, ScopedClock({None: VectorClock(vec)})
            )
        self.nc.all_engine_barrier()
        assert self.sems is not None
        popped = self.nc._tile_sem_poison_stack.pop()
        assert popped is self._sem_poison
        self.nc.clear_and_free_semaphores(list(self.sems.allocated().values()))
        self.nc.all_engine_barrier()

    tile.TileContext._drain_and_barrier = _drain_and_barrier_split
    tile.TileContext._drain_split_patched = True


def host_prep(x, norm_w, norm_b, qkv_w, qkv_b, proj_w, proj_b):
    """Host-side layout/dtype prep. Pure layout transforms + folding the
    1/sqrt(hd) attention scale into W_q/b_q (exact: 0.125 is a power of 2)."""
    x = np.ascontiguousarray(np.asarray(x, np.float32)).reshape(B, C, N)
    qkv_w = np.asarray(qkv_w, np.float32)
    qkv_b = np.asarray(qkv_b, np.float32)
    scale = float(HD) ** -0.5

    wqk = qkv_w[: 2 * C].copy()
    wqk[:C] *= scale
    bqk = qkv_b[: 2 * C].copy()
    bqk[:C] *= scale

    common = {
        "wqkT": np.ascontiguousarray(wqk.T).astype(BF16),            # [512,1024]
        "wvT": np.ascontiguousarray(qkv_w[2 * C :].T).astype(BF16),  # [512,512]
        "wpT": np.ascontiguousarray(np.asarray(proj_w, np.float32).T).astype(BF16),
        "bqk": np.ascontiguousarray(bqk.reshape(OC_QK, 128).T).astype(np.float32),
        "bv_row": qkv_b[2 * C :].reshape(1, C).astype(BF16),
        "bp": np.ascontiguousarray(
            np.asarray(proj_b, np.float32).reshape(CC, 128).T
        ).astype(np.float32),
        "nw": np.ascontiguousarray(
            np.asarray(norm_w, np.float32).reshape(CC, 128).T
        ).astype(np.float32),
        "nb": np.ascontiguousarray(
            np.asarray(norm_b, np.float32).reshape(CC, 128).T
        ).astype(np.float32),
        "gmat": _gmat(),
        "pselA": _psel(0, 32),
        "pselB": _psel(64, 96),
        "ones_col": np.ones((128, 1), BF16),
        "ones_row": np.ones((1, 128), BF16),
    }
    return common, [x[i] for i in range(B)]


def _gmat():
    g = np.zeros((128, 128), np.float32)
    per = 128 // 16  # channels per group = 16 -> 8 groups per 128-chunk
    for i in range(128):
        gi = i // 16
        g[i, gi * 16 : (gi + 1) * 16] = 1.0 / 16.0
    return g


def _psel(r0, r1):
    p = np.zeros((128, 128), np.float32)
    p[r0, 0:64] = 1.0
    p[r1, 64:128] = 1.0
    return p


def build_nc(unroll=1):
    _patch_tile()
    nc = bass.Bass()
    d = {}
    d["x"] = nc.declare_dram_parameter("x", [C, N], f32, isOutput=False)
    d["wqkT"] = nc.declare_dram_parameter("wqkT", [C, 2 * C], bf16, isOutput=False)
    d["wvT"] = nc.declare_dram_parameter("wvT", [C, C], bf16, isOutput=False)
    d["wpT"] = nc.declare_dram_parameter("wpT", [C, C], bf16, isOutput=False)
    d["bqk"] = nc.declare_dram_parameter("bqk", [128, OC_QK], f32, isOutput=False)
    d["bv_row"] = nc.declare_dram_parameter("bv_row", [1, C], bf16, isOutput=False)
    d["bp"] = nc.declare_dram_parameter("bp", [128, CC], f32, isOutput=False)
    d["nw"] = nc.declare_dram_parameter("nw", [128, CC], f32, isOutput=False)
    d["nb"] = nc.declare_dram_parameter("nb", [128, CC], f32, isOutput=False)
    d["gmat"] = nc.declare_dram_parameter("gmat", [128, 128], f32, isOutput=False)
    d["pselA"] = nc.declare_dram_parameter("pselA", [128, 128], f32, isOutput=False)
    d["pselB"] = nc.declare_dram_parameter("pselB", [128, 128], f32, isOutput=False)
    d["ones_col"] = nc.declare_dram_parameter("ones_col", [128, 1], bf16, isOutput=False)
    d["ones_row"] = nc.declare_dram_parameter("ones_row", [1, 128], bf16, isOutput=False)
    d["out"] = nc.declare_dram_parameter("out", [C, N], f32, isOutput=True)

    with tile.TileContext(nc) as tc:
        with (
            tc.tile_pool(name="sing", bufs=1) as sing,
            tc.tile_pool(name="gn", bufs=4) as gnp,
            tc.tile_pool(name="pp", bufs=10) as ppp,
            tc.tile_pool(name="rqp", bufs=2) as rqp,
            tc.tile_pool(name="psA", bufs=1, space="PSUM") as psA,
            tc.tile_pool(name="psB", bufs=2, space="PSUM") as psB,
            tc.tile_pool(name="psD", bufs=1, space="PSUM") as psD,
        ):
            # ---- constants (loaded once, shared by all unrolled iters) ----
            cst = {}
            for name, shape, dt in (
                ("wqkT", [128, CC, 2 * C], bf16),
                ("wvT", [128, CC, C], bf16),
                ("wpT", [128, CC, C], bf16),
                ("bqk", [128, OC_QK], f32),
                ("bv_row", [1, C], bf16),
                ("bp", [128, CC], f32),
                ("nw", [128, CC], f32),
                ("nb", [128, CC], f32),
                ("gmat", [128, 128], f32),
                ("pselA", [128, 128], f32),
                ("pselB", [128, 128], f32),
                ("ones_col", [128, 1], bf16),
                ("ones_row", [1, 128], bf16),
            ):
                t = sing.tile(shape, dt, tag=name, name=name)
                src = d[name]
                if len(shape) == 3:
                    # chunked weights: [c, o] dram -> [128, cc, o] sbuf
                    nc.sync.dma_start(
                        out=t, in_=src.rearrange("(cc p) o -> p cc o", p=128)
                    )
                else:
                    nc.sync.dma_start(out=t, in_=src[:])
                cst[name] = t
            eps_t = sing.tile([128, 1], f32, tag="eps", name="eps")
            nc.vector.memset(eps_t, EPS)

            s_ps = psD.tile([128, 512], f32, tag="s", name="s_ps")
            rb_ps = psD.tile([128, 512], f32, tag="rb", name="rb_ps")
            # stale psum rows multiply by zero weights in the psel matmul and
            # pass through reciprocal; they must be finite, never NaN.
            nc.vector.memset(s_ps, 1.0)
            nc.vector.memset(rb_ps, 1.0)

            for _ in range(unroll):
                _body(nc, tc, d, cst, sing, gnp, ppp, rqp, psA, psB, s_ps, rb_ps, eps_t)
    return nc


def _body_staged(nc, tc, d, cst, sing, gnp, ppp, rqp, psA, psB, s_ps, rb_ps, eps_t, stage):
    _body(nc, tc, d, cst, sing, gnp, ppp, rqp, psA, psB, s_ps, rb_ps, eps_t, stage=stage)


def _body(nc, tc, d, cst, sing, gnp, ppp, rqp, psA, psB, s_ps, rb_ps, eps_t, stage="full"):
    AF = mybir.ActivationFunctionType
    OP = mybir.AluOpType

    x4 = []
    for cc in range(CC):
        xt = sing.tile([128, N], f32, tag=f"x{cc}", name=f"x{cc}")
        nc.sync.dma_start(out=xt, in_=d["x"][cc * 128 : (cc + 1) * 128, :])
        x4.append(xt)

    # ---------------- GroupNorm -> xn (bf16) ----------------
    xn = []
    for cc in range(CC):
        stats = gnp.tile([128, 2, 6], f32, tag="stats", name="stats")
        for sub in range(2):
            nc.vector.bn_stats(
                out=stats[:, sub, :], in_=x4[cc][:, sub * 512 : (sub + 1) * 512]
            )
        mv = gnp.tile([128, 2], f32, tag="mv", name="mv")
        nc.vector.bn_aggr(out=mv, in_=stats)
        # ms = [mean, E[x^2]] per channel
        ms = gnp.tile([128, 2], f32, tag="ms", name="ms")
        nc.vector.tensor_copy(out=ms[:, 0:1], in_=mv[:, 0:1])
        nc.vector.scalar_tensor_tensor(
            out=ms[:, 1:2],
            in0=mv[:, 0:1],
            scalar=mv[:, 0:1],
            in1=mv[:, 1:2],
            op0=OP.mult,
            op1=OP.add,
        )
        gst_ps = psB.tile([128, 2], f32, tag="bank", name="gst_ps")
        nc.tensor.matmul(gst_ps, cst["gmat"], ms, start=True, stop=True)
        gst = gnp.tile([128, 2], f32, tag="gst", name="gst")
        nc.vector.tensor_copy(out=gst, in_=gst_ps)
        # negvar = mean_g^2 - E_g[x^2]  (rstd uses scale=-1 to flip sign)
        negvar = gnp.tile([128, 1], f32, tag="negvar", name="negvar")
        nc.vector.scalar_tensor_tensor(
            out=negvar,
            in0=gst[:, 0:1],
            scalar=gst[:, 0:1],
            in1=gst[:, 1:2],
            op0=OP.mult,
            op1=OP.subtract,
        )
        rstd = gnp.tile([128, 1], f32, tag="rstd", name="rstd")
        nc.scalar.activation(out=rstd, in_=negvar, func=AF.Sqrt, bias=eps_t, scale=-1.0)
        nc.vector.reciprocal(out=rstd, in_=rstd)
        aa = gnp.tile([128, 1], f32, tag="aa", name="aa")
        nc.vector.tensor_mul(out=aa, in0=rstd, in1=cst["nw"][:, cc : cc + 1])
        # bbn = mean_g*A - norm_b   (applied as x*A - bbn)
        bbn = gnp.tile([128, 1], f32, tag="bbn", name="bbn")
        nc.vector.scalar_tensor_tensor(
            out=bbn,
            in0=gst[:, 0:1],
            scalar=aa,
            in1=cst["nb"][:, cc : cc + 1],
            op0=OP.mult,
            op1=OP.subtract,
        )
        xnt = sing.tile([128, N], bf16, tag=f"xn{cc}", name=f"xn{cc}")
        nc.vector.tensor_scalar(
            out=xnt, in0=x4[cc], scalar1=aa, scalar2=bbn, op0=OP.mult, op1=OP.subtract
        )
        xn.append(xnt)

    if stage == "gn":
        return

    # ---------------- q, k generation ----------------
    qk = []
    for oc in range(OC_QK):
        qkt = sing.tile([128, N], bf16, tag=f"qk{oc}", name=f"qk{oc}")
        qk.append(qkt)
        for nci in range(NC):
            ps = psB.tile([128, 512], f32, tag="bank", name="bank")
            for cc in range(CC):
                nc.tensor.matmul(
                    ps,
                    cst["wqkT"][:, cc, oc * 128 : (oc + 1) * 128],
                    xn[cc][:, nci * 512 : (nci + 1) * 512],
                    start=(cc == 0),
                    stop=(cc == CC - 1),
                )
            nc.vector.tensor_scalar_add(
                out=qkt[:, nci * 512 : (nci + 1) * 512],
                in0=ps,
                scalar1=cst["bqk"][:, oc : oc + 1],
            )

    if stage == "qk":
        return

    # ---------------- V^T generation: V^T[n, vo] = xn^T @ wv^T ----------------
    vT = []
    for mci in range(MC):
        vt = sing.tile([128, C], bf16, tag=f"vT{mci}", name=f"vT{mci}")
        vT.append(vt)
        ps = psB.tile([128, 512], f32, tag="bank", name="bank")
        for cc in range(CC):
            nc.tensor.matmul(
                ps,
                xn[cc][:, mci * 128 : (mci + 1) * 128],
                cst["wvT"][:, cc, :],
                start=(cc == 0),
                stop=False,
            )
        nc.tensor.matmul(ps, cst["ones_row"], cst["bv_row"], start=False, stop=True)
        nc.vector.tensor_copy(out=vt, in_=ps)

    if stage == "vt":
        return

    # ---------------- attention ----------------
    o4 = [sing.tile([128, N], bf16, tag=f"o{cc}", name=f"o{cc}") for cc in range(CC)]

    for nci in range(NC):
        nwin = slice(nci * 512, (nci + 1) * 512)
        for q in range(2):  # quad of heads 4q..4q+3
            pp_tiles = {}
            for pi in range(2):  # pair within quad
                h0 = 4 * q + 2 * pi       # even head -> partitions 0:64
                h1 = h0 + 1               # odd head  -> partitions 64:128
                for g in range(4):
                    sg = psA.tile([128, 2048], f32, tag="sg", name="sg")
                    for sl, (h, mc) in enumerate(
                        [(h0, 2 * g), (h1, 2 * g), (h0, 2 * g + 1), (h1, 2 * g + 1)]
                    ):
                        hp = (h % 2) * 64
                        nc.tensor.matmul(
                            sg[:, sl * 512 : (sl + 1) * 512],
                            qk[4 + h // 2][hp : hp + 64, mc * 128 : (mc + 1) * 128],
                            qk[h // 2][hp : hp + 64, nwin],
                            start=True,
                            stop=True,
                            tile_position=(hp, 0),
                        )
                    pt = ppp.tile([128, 2048], bf16, tag="pp", name="pp")
                    nc.scalar.activation(out=pt, in_=sg, func=AF.Exp)
                    pp_tiles[(pi, g)] = pt

            if stage == "scores":
                continue

            att = [psB.tile([128, 512], f32, tag="bank", name="bank") for _ in range(2)]
            for mc in range(MC):
                g, par = mc // 2, mc % 2
                for pi in range(2):
                    h0 = 4 * q + 2 * pi
                    pt = pp_tiles[(pi, g)]
                    for hh in range(2):  # head within pair
                        sl = par * 2 + hh
                        nc.tensor.matmul(
                            att[pi][hh * 64 : (hh + 1) * 64, :],
                            vT[mc][:, (h0 + hh) * 64 : (h0 + hh + 1) * 64],
                            pt[:, sl * 512 : (sl + 1) * 512],
                            start=(mc == 0),
                            stop=(mc == MC - 1),
                            tile_position=(0, hh * 64),
                            skip_group_check=True,
                        )
                # denominators: 4-way col-tiled M=1 ones matmuls
                for pi in range(2):
                    pt = pp_tiles[(pi, g)]
                    for hh in range(2):
                        j = 2 * pi + hh
                        sl = par * 2 + hh
                        nc.tensor.matmul(
                            s_ps[32 * j : 32 * j + 1, :],
                            cst["ones_col"],
                            pt[:, sl * 512 : (sl + 1) * 512],
                            start=(mc == 0),
                            stop=(mc == MC - 1),
                            tile_position=(0, 32 * j),
                            skip_group_check=True,
                        )

            if stage == "att":
                continue

            rq = rqp.tile([128, 512], f32, tag="rq", name="rq")
            scr = rqp.tile([128, 512], f32, tag="scr", name="scr")
            if _RECIP_MODE == "approx":
                nc.vector.reciprocal_approx_accurate(out=rq, in_=s_ps, scratch=scr)
            else:
                nc.vector.reciprocal(out=rq, in_=s_ps)

            for pi in range(2):
                # O' exit (unnormalized) straight into the O channel tiles
                nc.vector.tensor_copy(out=o4[2 * q + pi][:, nwin], in_=att[pi])
                # broadcast r rows to 64-partition blocks, then normalize
                psel = cst["pselA"] if pi == 0 else cst["pselB"]
                nc.tensor.matmul(rb_ps, psel, rq, start=True, stop=True)
                nc.vector.tensor_tensor(
                    out=o4[2 * q + pi][:, nwin],
                    in0=o4[2 * q + pi][:, nwin],
                    in1=rb_ps,
                    op=mybir.AluOpType.mult,
                )
            if stage == "recip":
                continue

    if stage in ("scores", "att", "recip"):
        return

    # ---------------- proj + bias + residual ----------------
    for oc in range(CC):
        ob = sing.tile([128, N], f32, tag=f"ob{oc}", name=f"ob{oc}")
        for nci in range(NC):
            nwin = slice(nci * 512, (nci + 1) * 512)
            ps = psB.tile([128, 512], f32, tag="bank", name="bank")
            for cc in range(CC):
                nc.tensor.matmul(
                    ps,
                    cst["wpT"][:, cc, oc * 128 : (oc + 1) * 128],
                    o4[cc][:, nwin],
                    start=(cc == 0),
                    stop=(cc == CC - 1),
                )
            nc.vector.scalar_tensor_tensor(
                out=ob[:, nwin],
                in0=ps,
                scalar=cst["bp"][:, oc : oc + 1],
                in1=x4[oc][:, nwin],
                op0=OP.add,
                op1=OP.add,
            )
        nc.sync.dma_start(out=d["out"][oc * 128 : (oc + 1) * 128, :], in_=ob)


_BUILT = None


def kernel(**inputs):
    global _BUILT
    common, xs = host_prep(**inputs)
    if _BUILT is None:
        _BUILT = build_nc(unroll=1)
    nc = _BUILT
    in_maps = [dict(common, x=xs[i]) for i in range(B)]
    res = run_bass_kernel_spmd(nc, in_maps, core_ids=list(range(B)))
    out = np.stack([res.results[i]["out"] for i in range(B)], axis=0)
    return out.reshape(B, C, 32, 32).astype(np.float32)


# revision 14
# speedup vs baseline: 4.8866x; 4.8866x over previous
"""AttentionBlock (GroupNorm + MHSA + proj + residual) on 8 Trainium2 cores.

Sharding: pure data-parallel over batch (B=8, one batch element per core).
Per-core dataflow (C=512, N=H*W=1024, 8 heads, hd=64, 32 groups):
  1. GroupNorm stats via bn_stats per channel + PE group-mix matmul,
     affine folded with norm_w/norm_b -> xn (bf16).
  2. qkv: q,k via W^T matmul (q pre-scaled by hd^-0.5 on host);
     V^T computed directly as xn^T @ wv^T so the PV matmul needs no
     transposes on the critical path.
  3. Scores computed TRANSPOSED: S^T[m,n] = K^T Q (row-packed head
     pairs), exp on ACT (psum->sbuf bf16, 4-bank reads).
  4. O' = V^T-stationary matmul over P'^T (col-packed head pairs);
     softmax denominators ride as concurrent M=1 ones-matmuls into a
     shared psum bank (4-way col-tiled per quad).
  5. 1/s via DVE reciprocal_approx_accurate; partition-broadcast via a
     constant selection matmul; normalize during O' psum exit.
  6. proj matmul + bias + residual fused in one scalar_tensor_tensor.
"""

import numpy as np
import ml_dtypes

import concourse.bass as bass
import concourse.tile as tile
from concourse import mybir
from concourse.bass_utils import run_bass_kernel_spmd
from concourse.vector_clock import ScopedClock, VectorClock

f32 = mybir.dt.float32
bf16 = mybir.dt.bfloat16
BF16 = ml_dtypes.bfloat16

_RECIP_MODE = "plain"
_DENOM = True
B, C, N = 8, 512, 1024
NH, HD, G = 8, 64, 32
EPS = 1e-5
CC = C // 128  # 4 channel chunks
OC_QK = 8      # q+k output chunks (1024 rows)
NC = 2         # n in two 512-windows
MC = 8         # m in eight 128-chunks


def _split_multi_waits(bir_json):
    """This container's walrus build encodes at most one sync-wait command
    per TPB instruction. Engines execute in program order, so any extra
    waits can ride on NoOp instructions inserted immediately before the
    original instruction on the same engine (strictly more conservative
    ordering, semantics preserved)."""
    import orjson

    m = orjson.loads(bir_json)
    nop_id = [0]
    for fn in m.get("functions", []):
        for bb in fn.get("blocks", []):
            insts = bb.get("instructions", [])
            out = []
            for ins in insts:
                si = ins.get("sync_info") or {}
                waits = si.get("on_wait") or []
                eng = ins.get("engine", "Unassigned")
                if len(waits) > 1 and eng != "Unassigned":
                    for w in waits[:-1]:
                        nop_id[0] += 1
                        out.append(
                            {
                                "debug": ins.get("debug", 0),
                                "engine": eng,
                                "ins": [],
                                "outs": [],
                                "name": f"{ins['name']}-w{nop_id[0]}",
                                "opcode": "NoOp",
                                "sync_info": {"on_wait": [w]},
                            }
                        )
                    si = dict(si)
                    si["on_wait"] = [waits[-1]]
                    ins = dict(ins)
                    ins["sync_info"] = si
                out.append(ins)
            bb["instructions"] = out
    return orjson.dumps(m)


def _patch_tile():
    """This container's walrus accepts few sem-waits per instruction; split
    TileContext's kernel-tail drain into one drain per pending proc and
    hoist any remaining multi-waits onto NoOps at compile time."""
    if getattr(tile.TileContext, "_drain_split_patched", False):
        return

    from concourse import bass2jax, bass_utils

    orig_compile = bass_utils.compile_bir_kernel

    def compile_with_split(bir_json, tmpdir, neff_name="file.neff"):
        return orig_compile(_split_multi_waits(bir_json), tmpdir, neff_name=neff_name)

    bass_utils.compile_bir_kernel = compile_with_split
    bass2jax.compile_bir_kernel = compile_with_split

    def _drain_and_barrier_split(self, tick_clock, wait_clock):
        gc = tick_clock.global_clock
        ticks = list(gc)
        for p, t in enumerate(ticks):
            if t <= 0:
                continue
            vec = [0] * len(ticks)
            vec[p] = t
            drain_inst = self.nc.sync.drain()
            wait_clock.add_sem_waits(
                drain_inst.ins, ScopedClock({None: VectorClock(vec)})
            )
        self.nc.all_engine_barrier()
        assert self.sems is not None
        popped = self.nc._tile_sem_poison_stack.pop()
        assert popped is self._sem_poison
        self.nc.clear_and_free_semaphores(list(self.sems.allocated().values()))
        self.nc.all_engine_barrier()

    tile.TileContext._drain_and_barrier = _drain_and_barrier_split
    tile.TileContext._drain_split_patched = True


def host_prep(x, norm_w, norm_b, qkv_w, qkv_b, proj_w, proj_b):
    """Host-side layout/dtype prep. Pure layout transforms + folding the
    1/sqrt(hd) attention scale into W_q/b_q (exact: 0.125 is a power of 2)."""
    x = np.ascontiguousarray(np.asarray(x, np.float32)).reshape(B, C, N)
    qkv_w = np.asarray(qkv_w, np.float32)
    qkv_b = np.asarray(qkv_b, np.float32)
    scale = float(HD) ** -0.5

    wqk = qkv_w[: 2 * C].copy()
    wqk[:C] *= scale
    bqk = qkv_b[: 2 * C].copy()
    bqk[:C] *= scale

    common = {
        "wqkT": np.ascontiguousarray(wqk.T).astype(BF16),            # [512,1024]
        "wvT": np.ascontiguousarray(qkv_w[2 * C :].T).astype(BF16),  # [512,512]
        "wpT": np.ascontiguousarray(np.asarray(proj_w, np.float32).T).astype(BF16),
        "bqk": np.ascontiguousarray(bqk.reshape(OC_QK, 128).T).astype(np.float32),
        "bv_row": qkv_b[2 * C :].reshape(1, C).astype(BF16),
        "bp": np.ascontiguousarray(
            np.asarray(proj_b, np.float32).reshape(CC, 128).T
        ).astype(np.float32),
        "nw": np.ascontiguousarray(
            np.asarray(norm_w, np.float32).reshape(CC, 128).T
        ).astype(np.float32),
        "nb": np.ascontiguousarray(
            np.asarray(norm_b, np.float32).reshape(CC, 128).T
        ).astype(np.float32),
        "gmat": _gmat(),
        "pselA": _psel(0, 32),
        "pselB": _psel(64, 96),
        "ones_col": np.ones((128, 1), BF16),
        "ones_row": np.ones((1, 128), BF16),
    }
    return common, [x[i] for i in range(B)]


def _gmat():
    g = np.zeros((128, 128), np.float32)
    per = 128 // 16  # channels per group = 16 -> 8 groups per 128-chunk
    for i in range(128):
        gi = i // 16
        g[i, gi * 16 : (gi + 1) * 16] = 1.0 / (16.0 * 1024.0)
    return g


def _psel(r0, r1):
    p = np.zeros((128, 128), np.float32)
    p[r0, 0:64] = 1.0
    p[r1, 64:128] = 1.0
    return p


def build_nc(unroll=1):
    _patch_tile()
    nc = bass.Bass()
    d = {}
    d["x"] = nc.declare_dram_parameter("x", [C, N], f32, isOutput=False)
    d["wqkT"] = nc.declare_dram_parameter("wqkT", [C, 2 * C], bf16, isOutput=False)
    d["wvT"] = nc.declare_dram_parameter("wvT", [C, C], bf16, isOutput=False)
    d["wpT"] = nc.declare_dram_parameter("wpT", [C, C], bf16, isOutput=False)
    d["bqk"] = nc.declare_dram_parameter("bqk", [128, OC_QK], f32, isOutput=False)
    d["bv_row"] = nc.declare_dram_parameter("bv_row", [1, C], bf16, isOutput=False)
    d["bp"] = nc.declare_dram_parameter("bp", [128, CC], f32, isOutput=False)
    d["nw"] = nc.declare_dram_parameter("nw", [128, CC], f32, isOutput=False)
    d["nb"] = nc.declare_dram_parameter("nb", [128, CC], f32, isOutput=False)
    d["gmat"] = nc.declare_dram_parameter("gmat", [128, 128], f32, isOutput=False)
    d["pselA"] = nc.declare_dram_parameter("pselA", [128, 128], f32, isOutput=False)
    d["pselB"] = nc.declare_dram_parameter("pselB", [128, 128], f32, isOutput=False)
    d["ones_col"] = nc.declare_dram_parameter("ones_col", [128, 1], bf16, isOutput=False)
    d["ones_row"] = nc.declare_dram_parameter("ones_row", [1, 128], bf16, isOutput=False)
    d["out"] = nc.declare_dram_parameter("out", [C, N], f32, isOutput=True)

    with tile.TileContext(nc) as tc:
        with (
            tc.tile_pool(name="sing", bufs=1) as sing,
            tc.tile_pool(name="gn", bufs=4) as gnp,
            tc.tile_pool(name="pp", bufs=20) as ppp,
            tc.tile_pool(name="rqp", bufs=2) as rqp,
            tc.tile_pool(name="psA", bufs=2, space="PSUM") as psA,
            tc.tile_pool(name="psB", bufs=3, space="PSUM") as psB,
            tc.tile_pool(name="psD", bufs=1, space="PSUM") as psD,
        ):
            # ---- constants (loaded once, shared by all unrolled iters) ----
            cst = {}
            for name, shape, dt in (
                ("wqkT", [128, CC, 2 * C], bf16),
                ("wvT", [128, CC, C], bf16),
                ("wpT", [128, CC, C], bf16),
                ("bqk", [128, OC_QK], f32),
                ("bv_row", [1, C], bf16),
                ("bp", [128, CC], f32),
                ("nw", [128, CC], f32),
                ("nb", [128, CC], f32),
                ("gmat", [128, 128], f32),
                ("pselA", [128, 128], f32),
                ("pselB", [128, 128], f32),
                ("ones_col", [128, 1], bf16),
                ("ones_row", [1, 128], bf16),
            ):
                t = sing.tile(shape, dt, tag=name, name=name)
                src = d[name]
                if len(shape) == 3:
                    # chunked weights: [c, o] dram -> [128, cc, o] sbuf
                    nc.sync.dma_start(
                        out=t, in_=src.rearrange("(cc p) o -> p cc o", p=128)
                    )
                else:
                    nc.sync.dma_start(out=t, in_=src[:])
                cst[name] = t
            eps_t = sing.tile([128, 1], f32, tag="eps", name="eps")
            nc.vector.memset(eps_t, EPS)

            s_ps = psD.tile([128, 512], f32, tag="s", name="s_ps")
            # rb broadcasts reuse the sums bank after the reciprocal reads it.
            rb_ps = s_ps
            # stale psum rows multiply by zero weights in the psel matmul and
            # pass through reciprocal; they must be finite, never NaN.
            nc.vector.memset(s_ps, 1.0)

            for _ in range(unroll):
                _body(nc, tc, d, cst, sing, gnp, ppp, rqp, psA, psB, s_ps, rb_ps, eps_t)
    return nc


def _body_staged(nc, tc, d, cst, sing, gnp, ppp, rqp, psA, psB, s_ps, rb_ps, eps_t, stage):
    _body(nc, tc, d, cst, sing, gnp, ppp, rqp, psA, psB, s_ps, rb_ps, eps_t, stage=stage)


def _body(nc, tc, d, cst, sing, gnp, ppp, rqp, psA, psB, s_ps, rb_ps, eps_t, stage="full"):
    AF = mybir.ActivationFunctionType
    OP = mybir.AluOpType

    x4 = []
    for cc in range(CC):
        xt = sing.tile([128, N], f32, tag=f"x{cc}", name=f"x{cc}")
        nc.gpsimd.dma_start(out=xt, in_=d["x"][cc * 128 : (cc + 1) * 128, :])
        x4.append(xt)

    # ---------------- GroupNorm -> xn (bf16) ----------------
    xn = []
    sq = gnp.tile([128, N], f32, tag="sq", name="sq", bufs=2)
    for cc in range(CC):
        # ms = [sum(x), sum(x^2)] per channel; gmat folds the /16384
        ms = gnp.tile([128, 2], f32, tag="ms", name="ms")
        nc.vector.tensor_reduce(
            out=ms[:, 0:1], in_=x4[cc], axis=mybir.AxisListType.X, op=OP.add
        )
        nc.scalar.activation(
            out=sq, in_=x4[cc], func=AF.Square, accum_out=ms[:, 1:2]
        )
        gst_ps = psB.tile([128, 2], f32, tag="bank", name="gst_ps")
        nc.tensor.matmul(gst_ps, cst["gmat"], ms, start=True, stop=True)
        gst = gnp.tile([128, 2], f32, tag="gst", name="gst")
        nc.vector.tensor_copy(out=gst, in_=gst_ps)
        # negvar = mean_g^2 - E_g[x^2]  (rstd uses scale=-1 to flip sign)
        negvar = gnp.tile([128, 1], f32, tag="negvar", name="negvar")
        nc.vector.scalar_tensor_tensor(
            out=negvar,
            in0=gst[:, 0:1],
            scalar=gst[:, 0:1],
            in1=gst[:, 1:2],
            op0=OP.mult,
            op1=OP.subtract,
        )
        rstd = gnp.tile([128, 1], f32, tag="rstd", name="rstd")
        nc.scalar.activation(out=rstd, in_=negvar, func=AF.Sqrt, bias=eps_t, scale=-1.0)
        nc.vector.reciprocal(out=rstd, in_=rstd)
        aa = gnp.tile([128, 1], f32, tag="aa", name="aa")
        nc.vector.tensor_mul(out=aa, in0=rstd, in1=cst["nw"][:, cc : cc + 1])
        # bbn = mean_g*A - norm_b   (applied as x*A - bbn)
        bbn = gnp.tile([128, 1], f32, tag="bbn", name="bbn")
        nc.vector.scalar_tensor_tensor(
            out=bbn,
            in0=gst[:, 0:1],
            scalar=aa,
            in1=cst["nb"][:, cc : cc + 1],
            op0=OP.mult,
            op1=OP.subtract,
        )
        xnt = sing.tile([128, N], bf16, tag=f"xn{cc}", name=f"xn{cc}")
        nc.vector.tensor_scalar(
            out=xnt, in0=x4[cc], scalar1=aa, scalar2=bbn, op0=OP.mult, op1=OP.subtract
        )
        xn.append(xnt)

    if stage == "gn":
        return

    # ---------------- q, k generation ----------------
    qk = []
    for oc in range(OC_QK):
        qkt = sing.tile([128, N], bf16, tag=f"qk{oc}", name=f"qk{oc}")
        qk.append(qkt)
        for nci in range(NC):
            ps = psB.tile([128, 512], f32, tag="bank", name="bank")
            for cc in range(CC):
                nc.tensor.matmul(
                    ps,
                    cst["wqkT"][:, cc, oc * 128 : (oc + 1) * 128],
                    xn[cc][:, nci * 512 : (nci + 1) * 512],
                    start=(cc == 0),
                    stop=(cc == CC - 1),
                )
            nc.vector.tensor_scalar_add(
                out=qkt[:, nci * 512 : (nci + 1) * 512],
                in0=ps,
                scalar1=cst["bqk"][:, oc : oc + 1],
            )

    if stage == "qk":
        return

    # ---------------- V^T generation: V^T[n, vo] = xn^T @ wv^T ----------------
    vT = []
    for mci in range(MC):
        vt = sing.tile([128, C], bf16, tag=f"vT{mci}", name=f"vT{mci}")
        vT.append(vt)
        ps = psB.tile([128, 512], f32, tag="bank", name="bank")
        for cc in range(CC):
            nc.tensor.matmul(
                ps,
                xn[cc][:, mci * 128 : (mci + 1) * 128],
                cst["wvT"][:, cc, :],
                start=(cc == 0),
                stop=False,
            )
        nc.tensor.matmul(ps, cst["ones_row"], cst["bv_row"], start=False, stop=True)
        nc.vector.tensor_copy(out=vt, in_=ps)

    if stage == "vt":
        return

    # ---------------- attention ----------------
    o4 = [sing.tile([128, N], bf16, tag=f"o{cc}", name=f"o{cc}") for cc in range(CC)]

    for nci in range(NC):
        nwin = slice(nci * 512, (nci + 1) * 512)
        for q in range(2):  # quad of heads 4q..4q+3
            pp_tiles = {}
            for pi in range(2):  # pair within quad
                h0 = 4 * q + 2 * pi       # even head -> partitions 0:64
                h1 = h0 + 1               # odd head  -> partitions 64:128
                for mc in range(MC):
                    sg = psA.tile([128, 1024], f32, tag="sg", name="sg")
                    for sl, h in enumerate((h0, h1)):
                        hp = (h % 2) * 64
                        nc.tensor.matmul(
                            sg[:, sl * 512 : (sl + 1) * 512],
                            qk[4 + h // 2][hp : hp + 64, mc * 128 : (mc + 1) * 128],
                            qk[h // 2][hp : hp + 64, nwin],
                            start=True,
                            stop=True,
                            tile_position=(hp, 0),
                        )
                    pt = ppp.tile([128, 1024], bf16, tag="pp", name="pp")
                    nc.scalar.activation(out=pt, in_=sg, func=AF.Exp)
                    pp_tiles[(pi, mc)] = pt

            if stage == "scores":
                continue

            att = [psB.tile([128, 512], f32, tag="bank", name="bank") for _ in range(2)]
            for mc in range(MC):
                for pi in range(2):
                    h0 = 4 * q + 2 * pi
                    pt = pp_tiles[(pi, mc)]
                    for hh in range(2):  # head within pair
                        nc.tensor.matmul(
                            att[pi][hh * 64 : (hh + 1) * 64, :],
                            vT[mc][:, (h0 + hh) * 64 : (h0 + hh + 1) * 64],
                            pt[:, hh * 512 : (hh + 1) * 512],
                            start=(mc == 0),
                            stop=(mc == MC - 1),
                            tile_position=(0, hh * 64),
                            skip_group_check=True,
                        )
            # denominators after: 4-way col-tiled M=1 ones matmuls, so the
            # attnV stream is not blocked by the s_ps recip/rbc chain
            for mc in range(MC if _DENOM else 0):
                for pi in range(2):
                    pt = pp_tiles[(pi, mc)]
                    for hh in range(2):
                        j = 2 * pi + hh
                        nc.tensor.matmul(
                            s_ps[32 * j : 32 * j + 1, :],
                            cst["ones_col"],
                            pt[:, hh * 512 : (hh + 1) * 512],
                            start=(mc == 0),
                            stop=(mc == MC - 1),
                            tile_position=(0, 32 * j),
                            skip_group_check=True,
                        )

            if stage == "att" or not _DENOM:
                for pi in range(2):
                    nc.vector.tensor_copy(out=o4[2 * q + pi][:, nwin], in_=att[pi])
                continue

            rq = rqp.tile([128, 512], f32, tag="rq", name="rq")
            scr = rqp.tile([128, 512], f32, tag="scr", name="scr")
            if _RECIP_MODE == "approx":
                nc.vector.reciprocal_approx_accurate(out=rq, in_=s_ps, scratch=scr)
            else:
                nc.vector.reciprocal(out=rq, in_=s_ps)

            for pi in range(2):
                # O' exit (unnormalized) straight into the O channel tiles
                nc.vector.tensor_copy(out=o4[2 * q + pi][:, nwin], in_=att[pi])
                # broadcast r rows to 64-partition blocks, then normalize
                psel = cst["pselA"] if pi == 0 else cst["pselB"]
                nc.tensor.matmul(rb_ps, psel, rq, start=True, stop=True)
                nc.vector.tensor_tensor(
                    out=o4[2 * q + pi][:, nwin],
                    in0=o4[2 * q + pi][:, nwin],
                    in1=rb_ps,
                    op=mybir.AluOpType.mult,
                )
            if stage == "recip":
                continue

    if stage in ("scores", "att", "recip"):
        return

    # ---------------- proj + bias + residual (per n-window) ----------------
    for nci in range(NC):
        nwin = slice(nci * 512, (nci + 1) * 512)
        for oc in range(CC):
            ps = psB.tile([128, 512], f32, tag="bank", name="bank")
            for cc in range(CC):
                nc.tensor.matmul(
                    ps,
                    cst["wpT"][:, cc, oc * 128 : (oc + 1) * 128],
                    o4[cc][:, nwin],
                    start=(cc == 0),
                    stop=(cc == CC - 1),
                )
            ob = gnp.tile([128, 512], f32, tag="ob", name="ob", bufs=4)
            nc.vector.scalar_tensor_tensor(
                out=ob,
                in0=ps,
                scalar=cst["bp"][:, oc : oc + 1],
                in1=x4[oc][:, nwin],
                op0=OP.add,
                op1=OP.add,
            )
            nc.sync.dma_start(out=d["out"][oc * 128 : (oc + 1) * 128, nwin], in_=ob)


_BUILT = None


def kernel(**inputs):
    global _BUILT
    common, xs = host_prep(**inputs)
    if _BUILT is None:
        _BUILT = build_nc(unroll=1)
    nc = _BUILT
    in_maps = [dict(common, x=xs[i]) for i in range(B)]
    res = run_bass_kernel_spmd(nc, in_maps, core_ids=list(range(B)))
    out = np.stack([res.results[i]["out"] for i in range(B)], axis=0)
    return out.reshape(B, C, 32, 32).astype(np.float32)
